# revision 14
# baseline (speedup 1.0000x reference)
"""Trainium2 Bass kernel for nn_DATMambaLayer (DAT Mamba layer), 8 NeuronCores.

Self-contained: hardcodes all shapes/sharding. kernel(**inputs) accepts the
FULL inputs (same keys as reference.setup_inputs()) and returns the FULL
output tuple of 4 arrays.

Sharding (core k of 8):
- cm mamba (dm=512, di=1024): channel slice d_k = [128k, 128k+128); x_proj /
  out_proj contractions completed with 8-core AllReduces.
- sm mamba (dm=128, di=256): sample b = k//2, channel half h = k%2; x_proj
  and out_proj completed with pair AllReduces, then out AllGathered across
  the 4 samples.
- fusion: redundant on every core.
- reconstruct: everything at 14x14 (nearest-upsample commutes with all the
  pointwise ops, and BatchNorm batch stats are replication-invariant),
  channel-split at pw2 (cout/8 per core); each core nearest-upsamples and
  DMA-writes its channel slice of the full-res outputs.

Numeric mappings (ACT table sets restricted to sigmoid/erf, exp/ln, gelu):
- softplus(x) = ln(1+exp(x));  silu(x) = x*sigmoid(x)
- layernorm folded into in_proj: with W' = W diag(ln_w),
  xz = inv ⊙ (W'x) - (mean*inv) ⊙ u' + W ln_b, u' = rowsums(W')
- 1/sqrt(v+eps) via int32 bit-trick seed + 3 Newton iterations (DVE only)
- selective scan via tensor_tensor_scan (state = dA*state + dBu), dA zeroed
  at sequence starts so one scan covers concatenated sequences
- BatchNorm folded into the next 1x1 conv: lhsT scaled per input channel,
  constant term via a tiny N=1 matmul feeding the next ACT bias
"""
import os
import sys
from contextlib import ExitStack

sys.path.insert(0, '/opt/trn_rl_repo')

import numpy as np
import ml_dtypes

import concourse.bass as bass
import concourse.tile as tile
from concourse import bacc, mybir
from concourse.bass_utils import run_bass_kernel_spmd

F32 = mybir.dt.float32
BF16 = mybir.dt.bfloat16
I32 = mybir.dt.int32
AF = mybir.ActivationFunctionType
ALU = mybir.AluOpType

P = 128
NPC = 784
NPS = 4 * NPC        # 3136
EPS = 1e-5
NST = 16
SCALES = [16, 8, 4, 2]
COUTS = [64, 64, 128, 256]
CKS = [c // 8 for c in COUTS]   # per-core output channels: 8, 8, 16, 32
NC = 8
RSQRT_MAGIC = 0x5f3759df
DEBUG = bool(int(os.environ.get("KERNEL_DEBUG", "0")))

_CACHE = {}


def _bf(x):
    return np.ascontiguousarray(np.asarray(x, np.float32).astype(ml_dtypes.bfloat16))


def _f32(x):
    return np.ascontiguousarray(np.asarray(x, np.float32))


# ==========================================================================
# Device program
# ==========================================================================

def _mm_n(nc, psum, lhsT, rhs, start, stop, npix=NPC):
    off = 0
    while off < npix:
        size = min(512, npix - off)
        nc.tensor.matmul(psum[:, off:off + size], lhsT, rhs[:, off:off + size],
                         start=start, stop=stop)
        off += size


def _newton_rsqrt(nc, pool, var_ap, pdim, magic):
    """(pdim,1) f32 tile = 1/sqrt(var_ap + EPS)."""
    v = pool.tile([pdim, 1], F32, tag="nrt_v")
    nc.vector.tensor_scalar_add(v[:], var_ap, float(EPS))
    sh = pool.tile([pdim, 1], I32, tag="nrt_sh")
    nc.vector.tensor_scalar(sh[:], v[:].bitcast(I32), 1, None,
                            op0=ALU.arith_shift_right)
    y = pool.tile([pdim, 1], F32, tag="nrt_y")
    nc.vector.tensor_tensor(y[:].bitcast(I32), magic[:pdim, :], sh[:],
                            ALU.subtract)
    t = pool.tile([pdim, 1], F32, tag="nrt_t")
    for _ in range(3):
        nc.vector.tensor_tensor(t[:], y[:], y[:], ALU.mult)
        nc.vector.tensor_tensor(t[:], t[:], v[:], ALU.mult)
        nc.vector.tensor_scalar(t[:], t[:], -0.5, 1.5, op0=ALU.mult, op1=ALU.add)
        nc.vector.tensor_tensor(y[:], y[:], t[:], ALU.mult)
    return y


def _bn_stats(nc, pool, x_ap, pdim, tag):
    """Mean/var over a 784-wide free axis. Returns (pdim,2) tile."""
    st = pool.tile([pdim, 12], F32, tag=tag + "s")
    nc.vector.bn_stats(st[:, 0:6], x_ap[:, 0:392])
    nc.vector.bn_stats(st[:, 6:12], x_ap[:, 392:784])
    ag = pool.tile([pdim, 2], F32, tag=tag + "a")
    nc.vector.bn_aggr(ag[:], st[:])
    return ag


def _emit_mamba(nc, tc, ctx, pfx, cfg, io, g):
    """One mamba path for this core's slice. Returns debug tile dict."""
    kch, dtr = cfg['kch'], cfg['dtr']
    nseq, seqlen = cfg['nseq'], cfg['seqlen']
    xrows, mout = cfg['xrows'], cfg['mtiles_out']

    sb = ctx.enter_context(tc.tile_pool(name=pfx + "sb", bufs=1))
    sbn = ctx.enter_context(tc.tile_pool(name=pfx + "sbn", bufs=2))
    ps = g['ps']
    x_tiles = io['x_tiles']

    # ---- LN stats (ones-matmul on bf16 x) ----
    mean_ps = ps.tile([P, NPC], F32, tag="mm")
    for c in range(kch):
        _mm_n(nc, mean_ps, io['ones'], x_tiles[c][:], c == 0, c == kch - 1)
    msq_ps = ps.tile([P, NPC], F32, tag="mm")
    for c in range(kch):
        sq = sbn.tile([P, NPC], BF16, tag="sq")
        nc.scalar.activation(sq[:], x_tiles[c][:], AF.Square)
        _mm_n(nc, msq_ps, io['ones'], sq[:], c == 0, c == kch - 1)
    mean = sb.tile([P, NPC], F32, tag="mean")
    nc.scalar.activation(mean[:], mean_ps[:], AF.Copy)
    var = sb.tile([P, NPC], F32, tag="var")
    nc.vector.tensor_tensor(var[:], mean[:], mean[:], ALU.mult)
    nc.vector.tensor_tensor(var[:], msq_ps[:], var[:], ALU.subtract)
    inv = sb.tile([P, NPC], F32, tag="inv")
    nc.scalar.activation(inv[:], var[:], AF.Ln, bias=g['eps_ap'])
    nc.scalar.activation(inv[:], inv[:], AF.Exp, scale=-0.5)
    minv = sb.tile([P, NPC], F32, tag="minv")
    nc.vector.tensor_tensor(minv[:], mean[:], inv[:], ALU.mult)

    # ---- in_proj with LN fold ----
    w1 = []
    for c in range(kch):
        t = sb.tile([P, 256], BF16, tag=f"w1_{c}")
        nc.sync.dma_start(t[:], io['w1'][c])
        w1.append(t)
    upv = sb.tile([P, 2], F32, tag="upv")
    nc.sync.dma_start(upv[:], io['uprime'][:])
    vbv = sb.tile([P, 2], F32, tag="vbv")
    nc.sync.dma_start(vbv[:], io['vb'][:])

    xz = []
    for m in range(2):            # 0 = xs, 1 = z
        gp = ps.tile([P, NPC], F32, tag="mm")
        for c in range(kch):
            _mm_n(nc, gp, w1[c][:, m * P:(m + 1) * P], x_tiles[c][:],
                  c == 0, c == kch - 1)
        t = sb.tile([P, NPC], F32, tag=f"xz{m}")
        nc.vector.tensor_tensor(t[:], gp[:], inv[:], ALU.mult)
        nc.vector.scalar_tensor_tensor(t[:], minv[:], upv[:, m:m + 1], t[:],
                                       ALU.mult, ALU.add)
        xz.append(t)
    xs_t, z_t = xz

    # ---- conv + silu ----
    convw = sb.tile([P, 4], F32, tag="convw")
    nc.sync.dma_start(convw[:], io['convw'][:])
    convb = sb.tile([P, 1], F32, tag="convb")
    nc.sync.dma_start(convb[:], io['convb'][:])
    xp = sb.tile([P, nseq, seqlen + 3], F32, tag="xp")
    nc.vector.memset(xp[:, :, 0:3], 0.0)
    nc.scalar.activation(xp[:, :, 3:seqlen + 3],
                         xs_t[:].rearrange("p (a b) -> p a b", b=seqlen),
                         AF.Identity, bias=vbv[:, 0:1])
    cu = sb.tile([P, NPC], F32, tag="cu")
    cuv = cu[:].rearrange("p (a b) -> p a b", b=seqlen)
    nc.scalar.activation(cuv, xp[:, :, 3:seqlen + 3], AF.Identity,
                         bias=convb[:, 0:1], scale=convw[:, 3:4])
    for k in range(3):
        nc.vector.scalar_tensor_tensor(cuv, xp[:, :, k:k + seqlen],
                                       convw[:, k:k + 1], cuv, ALU.mult, ALU.add)
    usig = sbn.tile([P, NPC], F32, tag="usig")
    nc.scalar.activation(usig[:], cu[:], AF.Sigmoid)
    u = sb.tile([P, NPC], F32, tag="u")
    nc.vector.tensor_tensor(u[:], cu[:], usig[:], ALU.mult)

    # ---- gate t1 = z*sigmoid(z), with vb_z bias ----
    zf = sbn.tile([P, NPC], F32, tag="zf")
    nc.scalar.activation(zf[:], z_t[:], AF.Identity, bias=vbv[:, 1:2])
    zg = sbn.tile([P, NPC], F32, tag="zg")
    nc.scalar.activation(zg[:], z_t[:], AF.Sigmoid, bias=vbv[:, 1:2])
    t1 = sb.tile([P, NPC], F32, tag="t1")
    nc.vector.tensor_tensor(t1[:], zf[:], zg[:], ALU.mult)

    # ---- x_proj partial + allreduce ----
    xpT = sb.tile([P, xrows], BF16, tag="xpT")
    nc.sync.dma_start(xpT[:], io['xprojT'][:])
    ub = sb.tile([P, NPC], BF16, tag="ub")
    nc.scalar.activation(ub[:], u[:], AF.Copy)
    xd_ps = ps.tile([P, NPC], F32, tag="mm")
    _mm_n(nc, xd_ps[0:xrows, :], xpT[:], ub[:], True, True)
    xd_sb = sb.tile([xrows, NPC], F32, tag="xd")
    nc.scalar.activation(xd_sb[:], xd_ps[0:xrows, :], AF.Copy)
    nc.sync.dma_start(io['cc_in'][:], xd_sb[:])
    nc.gpsimd.collective_compute(
        "AllReduce", ALU.add, replica_groups=io['x_replica'],
        ins=[io['cc_in'][:]], outs=[io['cc_out'][:]])
    cc = io['cc_out']

    # ---- dt path ----
    dtr_b = sb.tile([dtr, NPC], BF16, tag="dtr_b")
    nc.gpsimd.dma_start(dtr_b[:], cc[0:dtr, :])
    dtpT = sb.tile([dtr, P], BF16, tag="dtpT")
    nc.sync.dma_start(dtpT[:], io['dtprojT'][:])
    dtb = sb.tile([P, 1], F32, tag="dtb")
    nc.sync.dma_start(dtb[:], io['dtb'][:])
    dt_ps = ps.tile([P, NPC], F32, tag="mm")
    _mm_n(nc, dt_ps, dtpT[:], dtr_b[:], True, True)
    dt = sb.tile([P, NPC], F32, tag="dt")
    nc.scalar.activation(dt[:], dt_ps[:], AF.Exp, bias=dtb[:, 0:1])
    nc.scalar.activation(dt[:], dt[:], AF.Ln, bias=1.0)

    Acols = sb.tile([P, NST], F32, tag="Acols")
    nc.sync.dma_start(Acols[:], io['A'][:])
    dtu = sb.tile([P, NPC], F32, tag="dtu")
    nc.vector.tensor_tensor(dtu[:], dt[:], u[:], ALU.mult)

    # ---- scan block ----
    pbuf = g['pbuf_pool'].tile([P, NST, NPC], F32, tag="pbuf")
    bc = ctx.enter_context(tc.tile_pool(name=pfx + "bc", bufs=4))
    dp = ctx.enter_context(tc.tile_pool(name=pfx + "dp", bufs=3))
    for n in range(NST):
        dA = dp.tile([P, NPC], F32, tag="dA")
        nc.scalar.activation(dA[:], dt[:], AF.Exp, scale=Acols[:, n:n + 1])
        nc.vector.memset(
            dA[:].rearrange("p (a b) -> p a b", b=seqlen)[:, :, 0:1], 0.0)
        Bb = bc.tile([P, NPC], F32, tag="Bb")
        nc.sync.dma_start(Bb[:], cc[dtr + n:dtr + n + 1, :].to_broadcast((P, NPC)))
        dBu = dp.tile([P, NPC], F32, tag="dBu")
        nc.vector.tensor_tensor(dBu[:], dtu[:], Bb[:], ALU.mult)
        nc.vector.tensor_tensor_scan(pbuf[:, n, :], dA[:], dBu[:], 0.0,
                                     ALU.mult, ALU.add)
        Cb = bc.tile([P, NPC], F32, tag="Cb")
        nc.sync.dma_start(Cb[:], cc[dtr + NST + n:dtr + NST + n + 1, :]
                          .to_broadcast((P, NPC)))
        nc.vector.tensor_tensor(pbuf[:, n, :], pbuf[:, n, :], Cb[:], ALU.mult)
    for half in (8, 4, 2, 1):
        nc.vector.tensor_tensor(pbuf[:, 0:half, :], pbuf[:, 0:half, :],
                                pbuf[:, half:2 * half, :], ALU.add)
    y = pbuf[:, 0, :]

    # ---- gate + out_proj partials ----
    Dv = sb.tile([P, 1], F32, tag="D")
    nc.sync.dma_start(Dv[:], io['D'][:])
    nc.vector.scalar_tensor_tensor(y, u[:], Dv[:, 0:1], y, ALU.mult, ALU.add)
    w_in = sb.tile([P, NPC], BF16, tag="w_in")
    nc.vector.tensor_tensor(w_in[:], y, t1[:], ALU.mult)

    for m in range(mout):
        oT = sbn.tile([P, P], BF16, tag="oT")
        nc.sync.dma_start(oT[:], io['outT'][m])
        op_ps = ps.tile([P, NPC], F32, tag="mm")
        _mm_n(nc, op_ps, oT[:], w_in[:], True, True)
        op_sb = sbn.tile([P, NPC], F32, tag="opsb")
        nc.scalar.activation(op_sb[:], op_ps[:], AF.Copy)
        nc.sync.dma_start(io['oc_in'][m * P:(m + 1) * P, :], op_sb[:])
    nc.gpsimd.collective_compute(
        "AllReduce", ALU.add, replica_groups=io['o_replica'],
        ins=[io['oc_in'][:]], outs=[io['oc_out'][:]])
    return dict(u=u, dt=dt, xd=xd_sb, t1=t1)


def _emit_tail(nc, tc, ctx, io, g):
    sb = ctx.enter_context(tc.tile_pool(name="tsb", bufs=1))
    sbn = ctx.enter_context(tc.tile_pool(name="tsbn", bufs=2))
    np_ = ctx.enter_context(tc.tile_pool(name="tnp", bufs=2))
    ps = g['ps']
    pst = ctx.enter_context(tc.tile_pool(name="tpst", bufs=2, space="PSUM"))
    dr = io['dr']
    s1f = io['s1f']
    magic = io['magic']

    # ---- m1s (+residual) and m1cp assembly ----
    m1s = sb.tile([P, NPS], F32, tag="m1s")
    for b in range(4):
        t = sbn.tile([P, NPC], F32, tag="m1sl")
        nc.sync.dma_start(t[:], io['m1s_dram'][b])
        nc.vector.tensor_tensor(m1s[:, b * NPC:(b + 1) * NPC], t[:],
                                s1f[:, b * NPC:(b + 1) * NPC], ALU.add)
    m1cp = sb.tile([P, NPS], F32, tag="m1cp")
    for j in range(4):
        t = sbn.tile([P, NPC], F32, tag="m1cl")
        nc.sync.dma_start(t[:], io['m1c_dram'][j * P:(j + 1) * P, :])
        dst = m1cp[:].rearrange("p (b q) -> p b q", q=NPC)[:, :, j * 196:(j + 1) * 196]
        src_s1 = s1f[:].rearrange("p (b q) -> p b q", q=NPC)[:, :, j * 196:(j + 1) * 196]
        nc.vector.tensor_tensor(dst, t[:].rearrange("p (b l) -> p b l", l=196),
                                src_s1, ALU.add)

    # ---- fusion ----
    m1cp_b = sb.tile([P, NPS], BF16, tag="m1cp_b")
    nc.scalar.activation(m1cp_b[:], m1cp[:], AF.Copy)
    m1s_b = sb.tile([P, NPS], BF16, tag="m1s_b")
    nc.scalar.activation(m1s_b[:], m1s[:], AF.Copy)
    fusT0 = sb.tile([P, 2], BF16, tag="fusT0")
    nc.sync.dma_start(fusT0[:], io['fusT'][0])
    fusT1 = sb.tile([P, 2], BF16, tag="fusT1")
    nc.sync.dma_start(fusT1[:], io['fusT'][1])
    fusb = sb.tile([2, 1], F32, tag="fusb")
    nc.sync.dma_start(fusb[:], io['fusb'][:])
    fw_sb = sb.tile([2, NPS], F32, tag="fw")
    for ni in range(7):
        o0 = ni * 448
        fp = pst.tile([2, 448], F32, tag="fps")
        nc.tensor.matmul(fp[:], fusT0[:], m1cp_b[:, o0:o0 + 448],
                         start=True, stop=False)
        nc.tensor.matmul(fp[:], fusT1[:], m1s_b[:, o0:o0 + 448],
                         start=False, stop=True)
        nc.scalar.activation(fw_sb[:, o0:o0 + 448], fp[:], AF.Sigmoid,
                             bias=fusb[:, 0:1])
    fw_dram = dr.tile([2, NPS], F32)
    nc.sync.dma_start(fw_dram[:], fw_sb[:])
    fused = sb.tile([P, NPS], F32, tag="fused")
    for hh in range(2):
        sl = slice(hh * (NPS // 2), (hh + 1) * (NPS // 2))
        f0 = sb.tile([P, NPS // 2], F32, tag="f0b")
        nc.sync.dma_start(f0[:], fw_dram[0:1, sl].to_broadcast((P, NPS // 2)))
        f1 = sb.tile([P, NPS // 2], F32, tag="f1b")
        nc.sync.dma_start(f1[:], fw_dram[1:2, sl].to_broadcast((P, NPS // 2)))
        nc.vector.tensor_tensor(f0[:], f0[:], m1cp[:, sl], ALU.mult)
        nc.vector.tensor_tensor(f1[:], f1[:], m1s[:, sl], ALU.mult)
        nc.vector.tensor_tensor(fused[:, sl], f0[:], f1[:], ALU.add)
    fused_b = sb.tile([P, NPS], BF16, tag="fused_b")
    nc.scalar.activation(fused_b[:], fused[:], AF.Copy)
    if DEBUG:
        nc.sync.dma_start(io['dbg']['m1s'][:], m1s[:])
        nc.sync.dma_start(io['dbg']['fused'][:], fused[:])

    # ---- reconstruct blocks ----
    for j, (s, ck) in enumerate(zip(SCALES, CKS)):
        fj = fused_b[:].rearrange("p (b q) -> p b q", q=NPC)[:, :, j * 196:(j + 1) * 196]
        dwp = sb.tile([P, 2], F32, tag="dw")
        nc.sync.dma_start(dwp[:], io['rdw'][j])
        g1 = sb.tile([P, NPC], BF16, tag="g1")
        nc.scalar.activation(g1[:].rearrange("p (b l) -> p b l", l=196), fj,
                             AF.Gelu, bias=dwp[:, 1:2], scale=dwp[:, 0:1])
        ag1 = _bn_stats(nc, np_, g1[:], P, "b1")
        inv1 = _newton_rsqrt(nc, np_, ag1[:, 1:2], P, magic)
        bn1 = sb.tile([P, 2], F32, tag="bn1")
        nc.sync.dma_start(bn1[:], io['rbn1'][j])
        s1a = np_.tile([P, 1], F32, tag="s1a")
        nc.vector.tensor_tensor(s1a[:], inv1[:], bn1[:, 0:1], ALU.mult)
        t1n = np_.tile([P, 1], F32, tag="t1n")
        nc.vector.scalar_tensor_tensor(t1n[:], ag1[:, 0:1], s1a[:, 0:1],
                                       bn1[:, 1:2], ALU.mult, ALU.subtract)
        t1nb = np_.tile([P, 1], BF16, tag="t1nb")
        nc.vector.tensor_copy(t1nb[:], t1n[:])

        pw1T = sb.tile([P, 512], BF16, tag="pw1T")
        nc.sync.dma_start(pw1T[:], io['rpw1T'][j])
        pw1s = sb.tile([P, 512], BF16, tag="pw1s")
        nc.scalar.activation(pw1s[:], pw1T[:], AF.Identity, scale=s1a[:, 0:1])
        c1_ps = pst.tile([P, 4], F32, tag="tiny")
        for m in range(4):
            nc.tensor.matmul(c1_ps[:, m:m + 1], pw1T[:, m * P:(m + 1) * P],
                             t1nb[:], start=True, stop=True)
        pw1b = sb.tile([P, 4], F32, tag="pw1b")
        nc.sync.dma_start(pw1b[:], io['rpw1b'][j])
        bias2 = sb.tile([P, 4], F32, tag="bias2")
        nc.vector.tensor_scalar(bias2[:], c1_ps[:], -1.0, None, op0=ALU.mult)
        nc.vector.tensor_tensor(bias2[:], bias2[:], pw1b[:], ALU.add)

        bn2 = sb.tile([P, 8], F32, tag="bn2")
        nc.sync.dma_start(bn2[:], io['rbn2'][j])
        g2s = []
        s2l, t2l = [], []
        for m in range(4):
            p2 = ps.tile([P, NPC], F32, tag="mm")
            _mm_n(nc, p2, pw1s[:, m * P:(m + 1) * P], g1[:], True, True)
            g2 = sb.tile([P, NPC], BF16, tag=f"g2_{m}")
            nc.scalar.activation(g2[:], p2[:], AF.Gelu, bias=bias2[:, m:m + 1])
            g2s.append(g2)
            ag2 = _bn_stats(nc, np_, g2[:], P, f"b2{m}")
            inv2 = _newton_rsqrt(nc, np_, ag2[:, 1:2], P, magic)
            s2a = np_.tile([P, 1], F32, tag="s2a")
            nc.vector.tensor_tensor(s2a[:], inv2[:], bn2[:, 2 * m:2 * m + 1],
                                    ALU.mult)
            t2n = np_.tile([P, 1], F32, tag="t2n")
            nc.vector.scalar_tensor_tensor(t2n[:], ag2[:, 0:1], s2a[:, 0:1],
                                           bn2[:, 2 * m + 1:2 * m + 2],
                                           ALU.mult, ALU.subtract)
            t2nb = np_.tile([P, 1], BF16, tag="t2nb")
            nc.vector.tensor_copy(t2nb[:], t2n[:])
            s2l.append(s2a)
            t2l.append(t2nb)

        c2_ps = pst.tile([P, 4], F32, tag="tiny")
        pw2sc = []
        for m in range(4):
            pw2T = sbn.tile([P, ck], BF16, tag=f"pw2T")
            nc.sync.dma_start(pw2T[:], io['rpw2T'][j][m])
            nc.tensor.matmul(c2_ps[0:ck, 0:1], pw2T[:], t2l[m][:],
                             start=(m == 0), stop=(m == 3))
            pw2s = sb.tile([P, ck], BF16, tag=f"pw2s{m}")
            nc.scalar.activation(pw2s[:], pw2T[:], AF.Identity,
                                 scale=s2l[m][:, 0:1])
            pw2sc.append(pw2s)
        pw2b = sb.tile([32, 1], F32, tag="pw2b")
        nc.sync.dma_start(pw2b[0:ck, :], io['rpw2b'][j])
        bias3 = sb.tile([32, 1], F32, tag="bias3")
        nc.vector.tensor_scalar(bias3[0:ck, :], c2_ps[0:ck, 0:1], -1.0, None,
                                op0=ALU.mult)
        nc.vector.tensor_tensor(bias3[0:ck, :], bias3[0:ck, :], pw2b[0:ck, :],
                                ALU.add)
        p3 = ps.tile([P, NPC], F32, tag="mm")
        for m in range(4):
            _mm_n(nc, p3[0:ck, :], pw2sc[m][:], g2s[m][:], m == 0, m == 3)
        g3 = sb.tile([32, NPC], F32, tag="g3")
        nc.scalar.activation(g3[0:ck, :], p3[0:ck, :], AF.Gelu,
                             bias=bias3[0:ck, 0:1])

        ag3 = _bn_stats(nc, np_, g3[0:ck, :], ck, "b3")
        inv3 = _newton_rsqrt(nc, np_, ag3[:, 1:2], ck, magic)
        bn3 = sb.tile([32, 2], F32, tag="bn3")
        nc.sync.dma_start(bn3[0:ck, :], io['rbn3'][j])
        s3a = np_.tile([32, 1], F32, tag="s3a")
        nc.vector.tensor_tensor(s3a[0:ck, :], inv3[:], bn3[0:ck, 0:1], ALU.mult)
        t3n = np_.tile([32, 1], F32, tag="t3n")
        nc.vector.scalar_tensor_tensor(t3n[0:ck, :], ag3[:, 0:1], s3a[0:ck, 0:1],
                                       bn3[0:ck, 1:2], ALU.mult, ALU.subtract)
        low = sb.tile([32, NPC], F32, tag="low")
        nc.vector.scalar_tensor_tensor(low[0:ck, :], g3[0:ck, :], s3a[0:ck, 0:1],
                                       t3n[0:ck, 0:1].to_broadcast((ck, NPC)),
                                       ALU.mult, ALU.subtract)
        if DEBUG:
            nc.sync.dma_start(io['dbg']['low'][j][0:ck, :], low[0:ck, :])

        # upsample: W-expand on DVE; H-replicate via DMA (large scales) or a
        # second DVE expand + single DMA (small scales)
        lw = low[0:ck, :].rearrange("c (b h w) -> c b h w", h=14, w=14)
        W = 14 * s
        if s >= 4:
            for b in range(4):
                wex = sb.tile([32, 14, W], F32, tag="wex")
                src = lw[:, b].unsqueeze(3).to_broadcast((ck, 14, 14, s))
                nc.vector.tensor_copy(
                    wex[0:ck, :, :].rearrange("c h (w r) -> c h w r", r=s), src)
                for hh in range(14):
                    nc.sync.dma_start(
                        io['outs'][j][b][:, hh * s:(hh + 1) * s, :],
                        wex[0:ck, hh, :].unsqueeze(1).to_broadcast((ck, s, W)))
        else:
            full = sb.tile([32, 4, W, W], F32, tag="full")
            for b in range(4):
                wex = sb.tile([32, 14, W], F32, tag="wex")
                src = lw[:, b].unsqueeze(3).to_broadcast((ck, 14, 14, s))
                nc.vector.tensor_copy(
                    wex[0:ck, :, :].rearrange("c h (w r) -> c h w r", r=s), src)
                nc.vector.tensor_copy(
                    full[0:ck, b].rearrange("c (h r) w -> c h r w", r=s),
                    wex[0:ck, :, :].unsqueeze(2).to_broadcast((ck, 14, s, W)))
            nc.sync.dma_start(
                io['outs'][j][:].rearrange("b c h w -> c b h w"),
                full[0:ck])


def build_program():
    nc = bacc.Bacc("TRN2", target_bir_lowering=False)
    dp = nc.declare_dram_parameter

    xc = dp("xc", [4, P, NPC], BF16, isOutput=False)
    s1f_d = dp("s1f", [P, NPS], F32, isOutput=False)
    s1b_d = dp("s1b", [P, NPC], BF16, isOutput=False)

    cw1 = dp("cw1", [4, P, 256], BF16, isOutput=False)
    cup = dp("cup", [P, 2], F32, isOutput=False)
    cvb = dp("cvb", [P, 2], F32, isOutput=False)
    cconvw = dp("cconvw", [P, 4], F32, isOutput=False)
    cconvb = dp("cconvb", [P, 1], F32, isOutput=False)
    cxprojT = dp("cxprojT", [P, 64], BF16, isOutput=False)
    cdtprojT = dp("cdtprojT", [32, P], BF16, isOutput=False)
    cdtb = dp("cdtb", [P, 1], F32, isOutput=False)
    cA = dp("cA", [P, NST], F32, isOutput=False)
    cD = dp("cD", [P, 1], F32, isOutput=False)
    coutT = dp("coutT", [4, P, P], BF16, isOutput=False)

    sw1 = dp("sw1", [1, P, 256], BF16, isOutput=False)
    sup = dp("sup", [P, 2], F32, isOutput=False)
    svb = dp("svb", [P, 2], F32, isOutput=False)
    sconvw = dp("sconvw", [P, 4], F32, isOutput=False)
    sconvb = dp("sconvb", [P, 1], F32, isOutput=False)
    sxprojT = dp("sxprojT", [P, 40], BF16, isOutput=False)
    sdtprojT = dp("sdtprojT", [8, P], BF16, isOutput=False)
    sdtb = dp("sdtb", [P, 1], F32, isOutput=False)
    sA = dp("sA", [P, NST], F32, isOutput=False)
    sD = dp("sD", [P, 1], F32, isOutput=False)
    soutT = dp("soutT", [1, P, P], BF16, isOutput=False)

    fusT = dp("fusT", [2, P, 2], BF16, isOutput=False)
    fusb = dp("fusb", [2, 1], F32, isOutput=False)

    rdw = dp("rdw", [4, P, 2], F32, isOutput=False)
    rbn1 = dp("rbn1", [4, P, 2], F32, isOutput=False)
    rpw1T = dp("rpw1T", [4, P, 512], BF16, isOutput=False)
    rpw1b = dp("rpw1b", [4, P, 4], F32, isOutput=False)
    rbn2 = dp("rbn2", [4, P, 8], F32, isOutput=False)
    rpw2T = [dp(f"rpw2T{j}", [4, P, CKS[j]], BF16, isOutput=False)
             for j in range(4)]
    rpw2b = [dp(f"rpw2b{j}", [CKS[j], 1], F32, isOutput=False) for j in range(4)]
    rbn3 = [dp(f"rbn3{j}", [CKS[j], 2], F32, isOutput=False) for j in range(4)]

    outs = [dp(f"o{j}", [4, CKS[j], 14 * s, 14 * s], F32, isOutput=True)
            for j, s in enumerate(SCALES)]

    dbg = {}
    if DEBUG:
        dbg['c_xd'] = dp("dbg_c_xd", [64, NPC], F32, isOutput=True)
        dbg['c_dt'] = dp("dbg_c_dt", [P, NPC], F32, isOutput=True)
        dbg['c_u'] = dp("dbg_c_u", [P, NPC], F32, isOutput=True)
        dbg['s_xd'] = dp("dbg_s_xd", [40, NPC], F32, isOutput=True)
        dbg['s_u'] = dp("dbg_s_u", [P, NPC], F32, isOutput=True)
        dbg['m1c'] = dp("dbg_m1c", [4 * P, NPC], F32, isOutput=True)
        dbg['m1s'] = dp("dbg_m1s", [P, NPS], F32, isOutput=True)
        dbg['fused'] = dp("dbg_fused", [P, NPS], F32, isOutput=True)
        dbg['low'] = [dp(f"dbg_low{j}", [32, NPC], F32, isOutput=True)
                      for j in range(4)]

    with tile.TileContext(nc) as tc, ExitStack() as octx:
        dr = octx.enter_context(tc.tile_pool(name="dram", bufs=1, space="DRAM"))
        gsb = octx.enter_context(tc.tile_pool(name="gsb", bufs=1))
        ps = octx.enter_context(tc.tile_pool(name="gps", bufs=2, space="PSUM"))
        g = dict(ps=ps)

        magic = gsb.tile([P, 1], I32, tag="magic")
        nc.vector.memset(magic[:], RSQRT_MAGIC)
        eps_t = gsb.tile([P, 1], F32, tag="epsc")
        nc.vector.memset(eps_t[:], float(EPS))
        g['eps_ap'] = eps_t[:, 0:1]
        ones_c = gsb.tile([P, P], BF16, tag="ones_c")
        nc.vector.memset(ones_c[:], 1.0 / 512.0)
        ones_s = gsb.tile([P, P], BF16, tag="ones_s")
        nc.vector.memset(ones_s[:], 1.0 / 128.0)

        xc_t = []
        for c in range(4):
            t = gsb.tile([P, NPC], BF16, tag=f"xc{c}")
            nc.sync.dma_start(t[:], xc[c])
            xc_t.append(t)
        s1b_t = gsb.tile([P, NPC], BF16, tag="s1b")
        nc.sync.dma_start(s1b_t[:], s1b_d[:])
        s1f_t = gsb.tile([P, NPS], F32, tag="s1f")
        nc.sync.dma_start(s1f_t[:], s1f_d[:])

        c_cc_in = dr.tile([64, NPC], F32)
        c_cc_out = dr.tile([64, NPC], F32)
        c_oc_in = dr.tile([4 * P, NPC], F32)
        c_oc_out = dr.tile([4 * P, NPC], F32)
        s_cc_in = dr.tile([40, NPC], F32)
        s_cc_out = dr.tile([40, NPC], F32)
        s_oc_in = dr.tile([P, NPC], F32)
        s_oc_out = dr.tile([P, NPC], F32)
        s_og_out = dr.tile([4, P, NPC], F32)

        with ExitStack() as mctx:
            pbuf_pool = mctx.enter_context(tc.tile_pool(name="pbuf", bufs=1))
            g['pbuf_pool'] = pbuf_pool
            with ExitStack() as cctx:
                cm_cfg = dict(kch=4, dtr=32, mtiles_out=4, nseq=4, seqlen=196,
                              xrows=64)
                cm_io = dict(
                    x_tiles=xc_t, w1=cw1, uprime=cup, vb=cvb, convw=cconvw,
                    convb=cconvb, xprojT=cxprojT, dtprojT=cdtprojT, dtb=cdtb,
                    A=cA, D=cD, outT=[coutT[m] for m in range(4)],
                    ones=ones_c[:],
                    cc_in=c_cc_in, cc_out=c_cc_out,
                    x_replica=[list(range(NC))],
                    oc_in=c_oc_in, oc_out=c_oc_out,
                    o_replica=[list(range(NC))])
                cm_res = _emit_mamba(nc, tc, cctx, "c", cm_cfg, cm_io, g)
                if DEBUG:
                    nc.sync.dma_start(dbg['c_xd'][:], cm_res['xd'][:])
                    nc.sync.dma_start(dbg['c_dt'][:], cm_res['dt'][:])
                    nc.sync.dma_start(dbg['c_u'][:], cm_res['u'][:])

            with ExitStack() as sctx:
                sm_cfg = dict(kch=1, dtr=8, mtiles_out=1, nseq=1, seqlen=NPC,
                              xrows=40)
                sm_io = dict(
                    x_tiles=[s1b_t], w1=sw1, uprime=sup, vb=svb, convw=sconvw,
                    convb=sconvb, xprojT=sxprojT, dtprojT=sdtprojT, dtb=sdtb,
                    A=sA, D=sD, outT=[soutT[0]],
                    ones=ones_s[:],
                    cc_in=s_cc_in, cc_out=s_cc_out,
                    x_replica=[[0, 1], [2, 3], [4, 5], [6, 7]],
                    oc_in=s_oc_in, oc_out=s_oc_out,
                    o_replica=[[0, 1], [2, 3], [4, 5], [6, 7]])
                sm_res = _emit_mamba(nc, tc, sctx, "s", sm_cfg, sm_io, g)
                nc.gpsimd.collective_compute(
                    "AllGather", ALU.bypass,
                    replica_groups=[[0, 2, 4, 6], [1, 3, 5, 7]],
                    ins=[s_oc_out[:]], outs=[s_og_out[:]])
                if DEBUG:
                    nc.sync.dma_start(dbg['s_xd'][:], sm_res['xd'][:])
                    nc.sync.dma_start(dbg['s_u'][:], sm_res['u'][:])

        if DEBUG:
            mdbg = gsb.tile([4 * P, NPC], F32, tag="mdbg")
            nc.sync.dma_start(mdbg[:], c_oc_out[:])
            nc.sync.dma_start(dbg['m1c'][:], mdbg[:])

        with ExitStack() as tctx:
            tail_io = dict(
                s1f=s1f_t, m1c_dram=c_oc_out, m1s_dram=s_og_out,
                fusT=fusT, fusb=fusb, rdw=rdw, rbn1=rbn1, rpw1T=rpw1T,
                rpw1b=rpw1b, rbn2=rbn2,
                rpw2T=[[rpw2T[j][m] for m in range(4)] for j in range(4)],
                rpw2b=rpw2b, rbn3=rbn3, outs=outs, magic=magic, dr=dr,
                dbg=dbg)
            _emit_tail(nc, tc, tctx, tail_io, g)

    nc.compile()
    return nc


# ==========================================================================
# Host side
# ==========================================================================

def pack_inputs(inputs):
    x1, x2, x3, x4 = (_f32(inputs[k]) for k in ('x1', 'x2', 'x3', 'x4'))
    cm = {k: _f32(v) for k, v in inputs['cm'].items()}
    sm = {k: _f32(v) for k, v in inputs['sm'].items()}
    rec = [{k: _f32(v) for k, v in r.items()} for r in inputs['rec']]
    cnw, cnb = _f32(inputs['cnorm_w']), _f32(inputs['cnorm_b'])
    snw, snb = _f32(inputs['snorm_w']), _f32(inputs['snorm_b'])
    fw, fb = _f32(inputs['fusion_w']), _f32(inputs['fusion_b'])

    xs4 = [x1, x2, x3, x4]
    xc_np = np.concatenate(xs4, axis=2).transpose(2, 0, 1).reshape(512, NPC)
    s1_np = np.stack(xs4, axis=1).transpose(3, 0, 1, 2).reshape(P, NPS)

    shared = dict(
        xc=_bf(xc_np.reshape(4, P, NPC)),
        s1f=_f32(s1_np),
        fusT=_bf(np.stack([fw[:, 0:P].T, fw[:, P:2 * P].T])),   # (2,128,2)
        fusb=_f32(fb.reshape(2, 1)),
        rdw=_f32(np.stack([np.stack([r['dw_w'], r['dw_b']], 1) for r in rec])),
        rbn1=_f32(np.stack([np.stack([r['bn1_g'], r['bn1_b']], 1) for r in rec])),
        rpw1T=_bf(np.stack([r['pw1_w'].T for r in rec])),        # (4,128,512)
        rpw1b=_f32(np.stack([r['pw1_b'].reshape(4, P).T for r in rec])),
        rbn2=_f32(np.stack([
            np.stack([r['bn2_g'].reshape(4, P), r['bn2_b'].reshape(4, P)], 2)
            .transpose(1, 0, 2).reshape(P, 8) for r in rec])),
    )

    # cm LN fold
    W1c = cm['in_proj_w'] * cnw[None, :]            # (2048, 512)
    vbc = cm['in_proj_w'] @ cnb                     # (2048,)
    upc = -W1c.sum(1)                               # (2048,)
    W1s = sm['in_proj_w'] * snw[None, :]
    vbs = sm['in_proj_w'] @ snb
    ups = -W1s.sum(1)
    Ac = -np.exp(cm['A_log'])
    As = -np.exp(sm['A_log'])

    per_core = []
    for k in range(NC):
        dlo, dhi = k * P, (k + 1) * P
        b, h = k // 2, k % 2
        slo, shi = h * P, (h + 1) * P
        rows_c = np.r_[dlo:dhi, 1024 + dlo:1024 + dhi]
        rows_s = np.r_[slo:shi, 256 + slo:256 + shi]
        m = dict(
            s1b=_bf(s1_np[:, b * NPC:(b + 1) * NPC]),
            cw1=_bf(np.stack([W1c[rows_c, c * P:(c + 1) * P].T
                              for c in range(4)])),
            cup=_f32(np.stack([upc[rows_c[:P]], upc[rows_c[P:]]], 1)),
            cvb=_f32(np.stack([vbc[rows_c[:P]], vbc[rows_c[P:]]], 1)),
            cconvw=_f32(cm['conv_w'][dlo:dhi]),
            cconvb=_f32(cm['conv_b'][dlo:dhi].reshape(P, 1)),
            cxprojT=_bf(cm['x_proj_w'][:, dlo:dhi].T),
            cdtprojT=_bf(cm['dt_proj_w'][dlo:dhi].T),
            cdtb=_f32(cm['dt_proj_b'][dlo:dhi].reshape(P, 1)),
            cA=_f32(Ac[dlo:dhi]),
            cD=_f32(cm['D'][dlo:dhi].reshape(P, 1)),
            coutT=_bf(np.stack([cm['out_proj_w'][mm * P:(mm + 1) * P, dlo:dhi].T
                                for mm in range(4)])),
            sw1=_bf(W1s[rows_s].T.reshape(1, P, 256)),
            sup=_f32(np.stack([ups[rows_s[:P]], ups[rows_s[P:]]], 1)),
            svb=_f32(np.stack([vbs[rows_s[:P]], vbs[rows_s[P:]]], 1)),
            sconvw=_f32(sm['conv_w'][slo:shi]),
            sconvb=_f32(sm['conv_b'][slo:shi].reshape(P, 1)),
            sxprojT=_bf(sm['x_proj_w'][:, slo:shi].T),
            sdtprojT=_bf(sm['dt_proj_w'][slo:shi].T),
            sdtb=_f32(sm['dt_proj_b'][slo:shi].reshape(P, 1)),
            sA=_f32(As[slo:shi]),
            sD=_f32(sm['D'][slo:shi].reshape(P, 1)),
            soutT=_bf(sm['out_proj_w'][:, slo:shi].T.reshape(1, P, P)),
        )
        for j, (ck, r) in enumerate(zip(CKS, rec)):
            rows = slice(k * ck, (k + 1) * ck)
            w2 = r['pw2_w'][rows]                    # (ck, 512)
            m[f"rpw2T{j}"] = _bf(w2.T.reshape(4, P, ck))
            m[f"rpw2b{j}"] = _f32(r['pw2_b'][rows].reshape(ck, 1))
            m[f"rbn3{j}"] = _f32(np.stack([r['bn3_g'][rows], r['bn3_b'][rows]], 1))
        m.update(shared)
        per_core.append(m)
    return per_core


def kernel(**inputs):
    global _CACHE
    if 'nc' not in _CACHE:
        _CACHE['nc'] = build_program()
    nc = _CACHE['nc']
    per_core = pack_inputs(inputs)
    res = run_bass_kernel_spmd(nc, per_core, core_ids=list(range(NC))).results
    outs = []
    for j in range(4):
        outs.append(np.concatenate([res[k][f"o{j}"] for k in range(NC)], axis=1))
    if DEBUG:
        _CACHE['raw'] = res
    return tuple(outs)
